# Initial kernel scaffold
#
"""ConcatCritic pair-MLP kernel for 8 Trainium2 NeuronCores.

scores[i, j] = MLP(concat(x_i, y_j)) with
MLP = Linear(256,512) -> ReLU -> Linear(512,512) -> ReLU -> Linear(512,1).

Sharding: pure data parallelism over the B^2 pair grid, split along the
x (row) index — each core gets 64 rows of x plus all of y and the full
(small) weight set, and produces a [64, 512] slab of the output.

Per-core dataflow (per x-row i):
  stage A (ACT):  h1T[h, j]   = relu(hyT[h, j] + (hx_i[h] + b1[h]))     4x [128,512]
  stage B (PE):   x2[j, k]    = h1_i @ W2   (fp32r matmuls, psum accum) 16x mm
  stage C (DVE):  s_i[j]      = sum_k relu(x2[j, k]) * W3[k]            4x scalar_tensor_tensor
with hxT = (x @ W1[:128]).T + b1 and hyT = (y @ W1[128:]).T computed once
in full fp32 during setup.  b2/b3 are applied exactly: b3 on the host,
b2 (when nonzero) via an extra K=1 matmul accumulated into psum.
"""

import numpy as np

B = 512
DX = 128
H = 512
N_CORES = 8
ROWS = B // N_CORES  # 64 x-rows per core
HC = H // 128  # 4 chunks of the hidden dim

_BUILT = {}  # (with_b2, use_f32r) -> bass.Bass


def _build(with_b2: bool, use_f32r: bool = True):
    import concourse.bass as bass
    import concourse.mybir as mybir
    from concourse.masks import make_identity
    from concourse.tile import TileContext

    F32 = mybir.dt.float32
    MMDT = mybir.dt.float32r if use_f32r else mybir.dt.float32
    Relu = mybir.ActivationFunctionType.Relu
    Alu = mybir.AluOpType

    def mm(ap):
        return ap.bitcast(MMDT)

    nc = bass.Bass()
    x_d = nc.declare_dram_parameter("x", [ROWS, DX], F32, isOutput=False)
    y_d = nc.declare_dram_parameter("y", [B, DX], F32, isOutput=False)
    w1_d = nc.declare_dram_parameter("W1", [2 * DX, H], F32, isOutput=False)
    b1_d = nc.declare_dram_parameter("b1", [H], F32, isOutput=False)
    w2_d = nc.declare_dram_parameter("W2", [H, H], F32, isOutput=False)
    w3_d = nc.declare_dram_parameter("W3", [H, 1], F32, isOutput=False)
    if with_b2:
        b2_d = nc.declare_dram_parameter("b2", [H], F32, isOutput=False)
    out_d = nc.declare_dram_parameter("out", [ROWS, B], F32, isOutput=True)

    with TileContext(nc) as tc:
        with (
            tc.tile_pool(name="consts", bufs=1) as cpool,
            tc.tile_pool(name="work", bufs=2) as wpool,
            tc.tile_pool(name="psum", bufs=8, space="PSUM") as ppool,
        ):
            # ---------------- constants & staging ----------------
            ident = cpool.tile([128, 128], F32, name="ident")
            make_identity(nc, ident)

            w1x = cpool.tile([128, H], F32, name="w1x")  # [d, h]
            nc.sync.dma_start(out=w1x[:], in_=w1_d[0:DX, :])
            w1y = cpool.tile([128, H], F32, name="w1y")  # [d, h]
            nc.sync.dma_start(out=w1y[:], in_=w1_d[DX : 2 * DX, :])

            w2sb = []  # [h-chunk, k] tiles
            for hc in range(HC):
                t = cpool.tile([128, H], F32, name=f"w2_{hc}")
                nc.sync.dma_start(out=t[:], in_=w2_d[hc * 128 : (hc + 1) * 128, :])
                w2sb.append(t)

            # W3 broadcast across partitions: w3b[p, k] = W3[k]
            w3b = cpool.tile([128, H], F32, name="w3b")
            nc.sync.dma_start(
                out=w3b[:],
                in_=w3_d[:, :].flatten().unsqueeze(0).to_broadcast([128, H]),
            )

            b1sb = cpool.tile([128, HC], F32, name="b1sb")  # [p, hc]
            nc.sync.dma_start(out=b1sb[:], in_=b1_d[:].rearrange("(c p) -> p c", p=128))

            if with_b2:
                b2row = cpool.tile([1, H], F32, name="b2row")
                nc.sync.dma_start(out=b2row[:], in_=b2_d[:].unsqueeze(0))
                ones1 = cpool.tile([1, 128], F32, name="ones1")
                nc.vector.memset(ones1[:], 1.0)

            # y staged as [p, c, d] = y[c*128+p, d], then PE-transposed to yT[d, j]
            ysb = cpool.tile([128, HC, DX], F32, name="ysb")
            nc.sync.dma_start(out=ysb[:], in_=y_d[:, :].rearrange("(c p) d -> p c d", p=128))
            xsb = cpool.tile([ROWS, DX], F32, name="xsb")
            nc.sync.dma_start(out=xsb[:], in_=x_d[:, :])

            yT = cpool.tile([128, B], F32, name="yT")  # [d, j]
            for c in range(HC):
                psyt = ppool.tile([128, 128], F32, name="psyt", tag="ps")
                nc.tensor.transpose(psyt[:], ysb[:, c, :], ident[:])
                nc.scalar.copy(out=yT[:, c * 128 : (c + 1) * 128], in_=psyt[:])

            xT = cpool.tile([128, ROWS], F32, name="xT")  # [d, i]
            psx = ppool.tile([128, ROWS], F32, name="psx", tag="ps")
            nc.tensor.transpose(psx[:], xsb[:], ident[0:ROWS, 0:ROWS])
            nc.scalar.copy(out=xT[:], in_=psx[:])

            # hxT[hc][h, i] = (x @ W1x).T + b1   (full fp32)
            hxT = []
            for hc in range(HC):
                pshx = ppool.tile([128, ROWS], F32, name="pshx", tag="ps")
                nc.tensor.matmul(
                    pshx[:], w1x[:, hc * 128 : (hc + 1) * 128], xT[:], start=True, stop=True
                )
                t = cpool.tile([128, ROWS], F32, name=f"hxT_{hc}")
                nc.vector.tensor_scalar_add(t[:], pshx[:], b1sb[:, hc : hc + 1])
                hxT.append(t)

            # hyT[hc][h, j] = (y @ W1y).T   (full fp32)
            hyT = []
            for hc in range(HC):
                pshy = ppool.tile([128, B], F32, name="pshy", tag="ps")
                nc.tensor.matmul(
                    pshy[:], w1y[:, hc * 128 : (hc + 1) * 128], yT[:], start=True, stop=True
                )
                t = cpool.tile([128, B], F32, name=f"hyT_{hc}")
                nc.scalar.copy(out=t[:], in_=pshy[:])
                hyT.append(t)

            # scores accumulated transposed: scoresT[jc][j, i]
            scoresT = [cpool.tile([128, ROWS], F32, name=f"scT_{jc}") for jc in range(HC)]

            # ---------------- main loop over x rows ----------------
            for i in range(ROWS):
                h1T = []
                for hc in range(HC):
                    t = wpool.tile([128, B], F32, name="h1T", tag="h1T", bufs=8)
                    nc.scalar.activation(
                        t[:], hyT[hc][:], Relu, bias=hxT[hc][:, i : i + 1], scale=1.0
                    )
                    h1T.append(t)
                for jc in range(HC):
                    ps2 = ppool.tile([128, B], F32, name="ps2", tag="ps")
                    for hc in range(HC):
                        nc.tensor.matmul(
                            ps2[:],
                            mm(h1T[hc][:, jc * 128 : (jc + 1) * 128]),
                            mm(w2sb[hc][:]),
                            start=(hc == 0),
                            stop=(hc == HC - 1 and not with_b2),
                        )
                    if with_b2:
                        nc.tensor.matmul(
                            ps2[:], mm(ones1[:]), mm(b2row[:]), start=False, stop=True
                        )
                    scr = wpool.tile([128, B], F32, name="scr", tag="scr", bufs=4)
                    nc.vector.scalar_tensor_tensor(
                        out=scr[:],
                        in0=ps2[:],
                        scalar=0.0,
                        in1=w3b[:],
                        op0=Alu.max,
                        op1=Alu.mult,
                        accum_out=scoresT[jc][:, i : i + 1],
                    )

            # ---------------- epilogue: transpose + store ----------------
            outsb = cpool.tile([ROWS, B], F32, name="outsb")
            for jc in range(HC):
                pst = ppool.tile([ROWS, 128], F32, name="pst", tag="ps")
                nc.tensor.transpose(pst[:], scoresT[jc][:], ident[:])
                nc.scalar.copy(out=outsb[:, jc * 128 : (jc + 1) * 128], in_=pst[:])
            nc.sync.dma_start(out=out_d[:, :], in_=outsb[:])

    return nc


def _get_nc(with_b2: bool, use_f32r: bool = True):
    key = (with_b2, use_f32r)
    if key not in _BUILT:
        _BUILT[key] = _build(with_b2, use_f32r)
    return _BUILT[key]


def _run(inputs: dict, trace: bool = False, use_f32r: bool = True, **spmd_kwargs):
    """Shard, execute on 8 cores, gather. Returns (scores, BassKernelResults)."""
    from concourse.bass_utils import run_bass_kernel_spmd

    x = np.ascontiguousarray(np.asarray(inputs["x"], dtype=np.float32))
    y = np.ascontiguousarray(np.asarray(inputs["y"], dtype=np.float32))
    W1 = np.ascontiguousarray(np.asarray(inputs["W1"], dtype=np.float32))
    b1 = np.ascontiguousarray(np.asarray(inputs["b1"], dtype=np.float32))
    W2 = np.ascontiguousarray(np.asarray(inputs["W2"], dtype=np.float32))
    b2 = np.ascontiguousarray(np.asarray(inputs["b2"], dtype=np.float32))
    W3 = np.ascontiguousarray(np.asarray(inputs["W3"], dtype=np.float32))
    b3 = np.asarray(inputs["b3"], dtype=np.float32)

    with_b2 = bool(np.any(b2))
    nc = _get_nc(with_b2, use_f32r)

    in_maps = []
    for c in range(N_CORES):
        m = {
            "x": np.ascontiguousarray(x[c * ROWS : (c + 1) * ROWS]),
            "y": y,
            "W1": W1,
            "b1": b1,
            "W2": W2,
            "W3": W3,
        }
        if with_b2:
            m["b2"] = b2
        in_maps.append(m)

    res = run_bass_kernel_spmd(
        nc, in_maps, core_ids=list(range(N_CORES)), trace=trace, **spmd_kwargs
    )
    out = np.concatenate([r["out"] for r in res.results], axis=0)
    if b3.size and np.any(b3):
        out = out + b3.reshape(-1)[0]
    return np.ascontiguousarray(out.astype(np.float32)), res


def kernel(**inputs) -> np.ndarray:
    out, _ = _run(inputs)
    return out


# revision 14
# speedup vs baseline: 1.0293x; 1.0293x over previous
"""ConcatCritic pair-MLP kernel for 8 Trainium2 NeuronCores.

scores[i, j] = MLP(concat(x_i, y_j)) with
MLP = Linear(256,512) -> ReLU -> Linear(512,512) -> ReLU -> Linear(512,1).

Sharding: pure data parallelism over the B^2 pair grid, split along the
x (row) index — each core gets 64 rows of x plus all of y and the full
(small) weight set, and produces a [64, 512] slab of the output.

The host passes x and y pre-transposed (xT [128,64] slab, yT [128,512]) and
receives the output in transposed chunk layout [4][128 j][64 i]; both
transposes are trivial numpy work. This keeps every PE instruction's operand
produced by exactly one engine so no matmul needs more than one semaphore
wait (walrus's fused-weight-load matmuls reject multi-wait configurations).

Per-core dataflow (per x-row i):
  stage A (DVE+ACT): h1T[h, j] = relu(hyT[h, j] + (hx_i[h] + b1[h]))     4x [128,512]
  stage B (PE):      x2[j, k]  = h1_i @ W2   (fp32r matmuls, psum accum) 16x mm
  stage C (DVE):     s_i[j]    = sum_k relu(x2[j, k]) * W3[k]            4x scalar_tensor_tensor
with hxT = (x @ W1[:128]).T + b1 and hyT = (y @ W1[128:]).T computed once at
setup. b3 is applied on the host; b2 (nonzero only) via an extra K=1 matmul.
"""

import numpy as np

B = 512
DX = 128
H = 512
N_CORES = 8
ROWS = B // N_CORES  # 64 x-rows per core
HC = H // 128  # 4 chunks of the hidden dim

_BUILT = {}  # (with_b2, use_f32r) -> bass.Bass


def _build(with_b2: bool, use_f32r: bool = True):
    import concourse.mybir as mybir
    from concourse.bacc import Bacc
    from concourse.tile import TileContext

    F32 = mybir.dt.float32
    # fp32r = fp32 rounded to the PE's fast-path input precision: 1 cycle/row
    # instead of 4 at N>=256. The BIR verifier requires every producer of an
    # fp32r matmul operand to write the buffer AS float32r (round-on-write),
    # so all matmul operand tiles are allocated with this dtype and filled by
    # ACT/DVE copies.
    MMDT = mybir.dt.float32r if use_f32r else mybir.dt.float32
    Relu = mybir.ActivationFunctionType.Relu
    Alu = mybir.AluOpType

    # Bacc (not raw Bass): its compile pipeline splits multi-semaphore waits
    # into event-semaphore chains — TRN2 engine instructions accept only one
    # sync wait, which walrus otherwise rejects ("Too many sync wait
    # commands").
    nc = Bacc()
    xT_d = nc.declare_dram_parameter("xT", [DX, ROWS], F32, isOutput=False)
    yT_d = nc.declare_dram_parameter("yT", [DX, B], F32, isOutput=False)
    w1_d = nc.declare_dram_parameter("W1", [2 * DX, H], F32, isOutput=False)
    b1_d = nc.declare_dram_parameter("b1", [H], F32, isOutput=False)
    w2_d = nc.declare_dram_parameter("W2", [H, H], F32, isOutput=False)
    w3_d = nc.declare_dram_parameter("W3", [H, 1], F32, isOutput=False)
    if with_b2:
        b2_d = nc.declare_dram_parameter("b2", [H], F32, isOutput=False)
    out_d = nc.declare_dram_parameter("outT", [HC, 128, ROWS], F32, isOutput=True)

    with TileContext(nc) as tc:
        with (
            tc.tile_pool(name="consts", bufs=1) as cpool,
            tc.tile_pool(name="work", bufs=2) as wpool,
            tc.tile_pool(name="psum", bufs=8, space="PSUM") as ppool,
        ):
            # ---------------- staging DMAs (all fp32) ----------------
            # y-path first: the main loop can't start until hyT is ready, so
            # its DMAs and round-copies lead the schedule.
            yT_s = cpool.tile([DX, B], F32, name="yT_s")
            nc.sync.dma_start(out=yT_s[:], in_=yT_d[:, :])
            w1y_s = cpool.tile([DX, H], F32, name="w1y_s")
            nc.sync.dma_start(out=w1y_s[:], in_=w1_d[DX : 2 * DX, :])
            xT_s = cpool.tile([DX, ROWS], F32, name="xT_s")
            nc.sync.dma_start(out=xT_s[:], in_=xT_d[:, :])
            w1x_s = cpool.tile([DX, H], F32, name="w1x_s")
            nc.sync.dma_start(out=w1x_s[:], in_=w1_d[0:DX, :])
            b1sb = cpool.tile([128, HC], F32, name="b1sb")  # [p, hc]
            nc.sync.dma_start(out=b1sb[:], in_=b1_d[:].rearrange("(c p) -> p c", p=128))
            # W3 broadcast across partitions: w3b[p, k] = W3[k]
            w3b = cpool.tile([128, H], F32, name="w3b")
            nc.sync.dma_start(
                out=w3b[:],
                in_=w3_d[:, :].flatten().unsqueeze(0).to_broadcast([128, H]),
            )
            w2_s = []
            for hc in range(HC):
                t = cpool.tile([128, H], F32, name=f"w2s_{hc}")
                nc.sync.dma_start(out=t[:], in_=w2_d[hc * 128 : (hc + 1) * 128, :])
                w2_s.append(t)

            # ---------------- round-copies to the matmul dtype (ACT) ----------
            yT = cpool.tile([DX, B], MMDT, name="yT")
            nc.scalar.copy(out=yT[:], in_=yT_s[:])
            w1y = cpool.tile([DX, H], MMDT, name="w1y")
            nc.scalar.copy(out=w1y[:], in_=w1y_s[:])
            xT = cpool.tile([DX, ROWS], MMDT, name="xT")
            nc.scalar.copy(out=xT[:], in_=xT_s[:])
            w1x = cpool.tile([DX, H], MMDT, name="w1x")
            nc.scalar.copy(out=w1x[:], in_=w1x_s[:])
            w2sb = []
            for hc in range(HC):
                t = cpool.tile([128, H], MMDT, name=f"w2_{hc}")
                nc.scalar.copy(out=t[:], in_=w2_s[hc][:])
                w2sb.append(t)
            if with_b2:
                b2_s = cpool.tile([1, H], F32, name="b2_s")
                nc.sync.dma_start(out=b2_s[:], in_=b2_d[:].unsqueeze(0))
                b2row = cpool.tile([1, H], MMDT, name="b2row")
                nc.scalar.copy(out=b2row[:], in_=b2_s[:])
                ones1 = cpool.tile([1, 128], MMDT, name="ones1")
                nc.vector.memset(ones1[:], 1.0)

            # hxT[hc][h, i] = (x @ W1x).T + b1
            hxT = []
            for hc in range(HC):
                pshx = ppool.tile([128, ROWS], F32, name="pshx", tag="ps")
                nc.tensor.matmul(
                    pshx[:], w1x[:, hc * 128 : (hc + 1) * 128], xT[:], start=True, stop=True
                )
                t = cpool.tile([128, ROWS], F32, name=f"hxT_{hc}")
                nc.vector.tensor_scalar_add(t[:], pshx[:], b1sb[:, hc : hc + 1])
                hxT.append(t)

            # hyT[hc][h, j] = (y @ W1y).T
            hyT = []
            for hc in range(HC):
                pshy = ppool.tile([128, B], F32, name="pshy", tag="ps")
                nc.tensor.matmul(
                    pshy[:], w1y[:, hc * 128 : (hc + 1) * 128], yT[:], start=True, stop=True
                )
                t = cpool.tile([128, B], F32, name=f"hyT_{hc}")
                nc.vector.tensor_copy(out=t[:], in_=pshy[:])
                hyT.append(t)

            # scores accumulated transposed: scoresT[jc][j, i]
            scoresT = [cpool.tile([128, ROWS], F32, name=f"scT_{jc}") for jc in range(HC)]

            # ---------------- main loop over x rows ----------------
            for i in range(ROWS):
                h1T = []
                for hc in range(HC):
                    # ACT: relu(hyT + hx_i). All of stage A lives on ACT so the
                    # DVE has headroom for the stage-C fused reduce — DVE is
                    # otherwise the bottleneck engine (measured 92% busy).
                    t = wpool.tile([128, B], MMDT, name="h1T", tag="h1T", bufs=8)
                    nc.scalar.activation(
                        t[:], hyT[hc][:], Relu, bias=hxT[hc][:, i : i + 1], scale=1.0
                    )
                    h1T.append(t)
                for jc in range(HC):
                    ps2 = ppool.tile([128, B], F32, name="ps2", tag="ps")
                    for hc in range(HC):
                        nc.tensor.matmul(
                            ps2[:],
                            h1T[hc][:, jc * 128 : (jc + 1) * 128],
                            w2sb[hc][:],
                            start=(hc == 0),
                            stop=(hc == HC - 1 and not with_b2),
                        )
                    if with_b2:
                        nc.tensor.matmul(
                            ps2[:], ones1[:], b2row[:], start=False, stop=True
                        )
                    # DVE: scr = relu(ps2) * W3_bcast; scoresT col = sum_k scr
                    scr = wpool.tile([128, B], F32, name="scr", tag="scr", bufs=4)
                    nc.vector.scalar_tensor_tensor(
                        out=scr[:],
                        in0=ps2[:],
                        scalar=0.0,
                        in1=w3b[:],
                        op0=Alu.max,
                        op1=Alu.mult,
                        accum_out=scoresT[jc][:, i : i + 1],
                    )

            # ---------------- store (host un-transposes) ----------------
            for jc in range(HC):
                nc.sync.dma_start(out=out_d[jc, :, :], in_=scoresT[jc][:])

    nc.finalize()  # runs the Bacc pass pipeline (wait splitting etc.)
    return nc


def _get_nc(with_b2: bool, use_f32r: bool = True):
    key = (with_b2, use_f32r)
    if key not in _BUILT:
        _BUILT[key] = _build(with_b2, use_f32r)
    return _BUILT[key]


def _run(inputs: dict, trace: bool = False, use_f32r: bool = True, **spmd_kwargs):
    """Shard, execute on 8 cores, gather. Returns (scores, BassKernelResults)."""
    from concourse.bass_utils import run_bass_kernel_spmd

    x = np.asarray(inputs["x"], dtype=np.float32)
    y = np.asarray(inputs["y"], dtype=np.float32)
    W1 = np.ascontiguousarray(np.asarray(inputs["W1"], dtype=np.float32))
    b1 = np.ascontiguousarray(np.asarray(inputs["b1"], dtype=np.float32))
    W2 = np.ascontiguousarray(np.asarray(inputs["W2"], dtype=np.float32))
    b2 = np.ascontiguousarray(np.asarray(inputs["b2"], dtype=np.float32))
    W3 = np.ascontiguousarray(np.asarray(inputs["W3"], dtype=np.float32))
    b3 = np.asarray(inputs["b3"], dtype=np.float32)

    with_b2 = bool(np.any(b2))
    nc = _get_nc(with_b2, use_f32r)

    yT = np.ascontiguousarray(y.T)
    in_maps = []
    for c in range(N_CORES):
        m = {
            "xT": np.ascontiguousarray(x[c * ROWS : (c + 1) * ROWS].T),
            "yT": yT,
            "W1": W1,
            "b1": b1,
            "W2": W2,
            "W3": W3,
        }
        if with_b2:
            m["b2"] = b2
        in_maps.append(m)

    res = run_bass_kernel_spmd(
        nc, in_maps, core_ids=list(range(N_CORES)), trace=trace, **spmd_kwargs
    )
    # outT[jc, j, i] -> scores_slab[i, jc*128 + j]
    slabs = [
        np.transpose(r["outT"], (2, 0, 1)).reshape(ROWS, B) for r in res.results
    ]
    out = np.concatenate(slabs, axis=0)
    if b3.size and np.any(b3):
        out = out + b3.reshape(-1)[0]
    return np.ascontiguousarray(out.astype(np.float32)), res


def kernel(**inputs) -> np.ndarray:
    out, _ = _run(inputs)
    return out


# revision 16
# speedup vs baseline: 1.0299x; 1.0006x over previous
"""ConcatCritic pair-MLP kernel for 8 Trainium2 NeuronCores.

scores[i, j] = MLP(concat(x_i, y_j)) with
MLP = Linear(256,512) -> ReLU -> Linear(512,512) -> ReLU -> Linear(512,1).

Sharding: pure data parallelism over the B^2 pair grid, split along the
x (row) index — each core gets 64 rows of x plus all of y and the full
(small) weight set, and produces a [64, 512] slab of the output.

The host passes x and y pre-transposed (xT [128,64] slab, yT [128,512]) and
receives the output in transposed chunk layout [4][128 j][64 i]; both
transposes are trivial numpy work. This keeps every PE instruction's operand
produced by exactly one engine so no matmul needs more than one semaphore
wait (walrus's fused-weight-load matmuls reject multi-wait configurations).

Per-core dataflow (per x-row i):
  stage A (DVE+ACT): h1T[h, j] = relu(hyT[h, j] + (hx_i[h] + b1[h]))     4x [128,512]
  stage B (PE):      x2[j, k]  = h1_i @ W2   (fp32r matmuls, psum accum) 16x mm
  stage C (DVE):     s_i[j]    = sum_k relu(x2[j, k]) * W3[k]            4x scalar_tensor_tensor
with hxT = (x @ W1[:128]).T + b1 and hyT = (y @ W1[128:]).T computed once at
setup. b3 is applied on the host; b2 (nonzero only) via an extra K=1 matmul.
"""

import numpy as np

B = 512
DX = 128
H = 512
N_CORES = 8
ROWS = B // N_CORES  # 64 x-rows per core
HC = H // 128  # 4 chunks of the hidden dim

_BUILT = {}  # (with_b2, use_f32r) -> bass.Bass


def _build(with_b2: bool, use_f32r: bool = True):
    import concourse.mybir as mybir
    from concourse.bacc import Bacc
    from concourse.tile import TileContext

    F32 = mybir.dt.float32
    # fp32r = fp32 rounded to the PE's fast-path input precision: 1 cycle/row
    # instead of 4 at N>=256. The BIR verifier requires every producer of an
    # fp32r matmul operand to write the buffer AS float32r (round-on-write),
    # so all matmul operand tiles are allocated with this dtype and filled by
    # ACT/DVE copies.
    MMDT = mybir.dt.float32r if use_f32r else mybir.dt.float32
    Relu = mybir.ActivationFunctionType.Relu
    Alu = mybir.AluOpType

    # Bacc (not raw Bass): its compile pipeline splits multi-semaphore waits
    # into event-semaphore chains — TRN2 engine instructions accept only one
    # sync wait, which walrus otherwise rejects ("Too many sync wait
    # commands").
    nc = Bacc()
    xT_d = nc.declare_dram_parameter("xT", [DX, ROWS], F32, isOutput=False)
    yT_d = nc.declare_dram_parameter("yT", [DX, B], F32, isOutput=False)
    w1_d = nc.declare_dram_parameter("W1", [2 * DX, H], F32, isOutput=False)
    b1_d = nc.declare_dram_parameter("b1", [H], F32, isOutput=False)
    w2_d = nc.declare_dram_parameter("W2", [H, H], F32, isOutput=False)
    w3_d = nc.declare_dram_parameter("W3", [H, 1], F32, isOutput=False)
    if with_b2:
        b2_d = nc.declare_dram_parameter("b2", [H], F32, isOutput=False)
    out_d = nc.declare_dram_parameter("outT", [HC, 128, ROWS], F32, isOutput=True)

    with TileContext(nc) as tc:
        with (
            tc.tile_pool(name="consts", bufs=1) as cpool,
            tc.tile_pool(name="work", bufs=2) as wpool,
            tc.tile_pool(name="psum", bufs=8, space="PSUM") as ppool,
        ):
            # ---------------- staging DMAs (all fp32) ----------------
            # y-path first: the main loop can't start until hyT is ready, so
            # its DMAs and round-copies lead the schedule.
            yT_s = cpool.tile([DX, B], F32, name="yT_s")
            nc.sync.dma_start(out=yT_s[:], in_=yT_d[:, :])
            w1y_s = cpool.tile([DX, H], F32, name="w1y_s")
            nc.sync.dma_start(out=w1y_s[:], in_=w1_d[DX : 2 * DX, :])
            xT_s = cpool.tile([DX, ROWS], F32, name="xT_s")
            nc.sync.dma_start(out=xT_s[:], in_=xT_d[:, :])
            w1x_s = cpool.tile([DX, H], F32, name="w1x_s")
            nc.sync.dma_start(out=w1x_s[:], in_=w1_d[0:DX, :])
            b1sb = cpool.tile([128, HC], F32, name="b1sb")  # [p, hc]
            nc.sync.dma_start(out=b1sb[:], in_=b1_d[:].rearrange("(c p) -> p c", p=128))
            # W2 chunks ride other engines' DGE rings so they don't serialize
            # behind the y/x path on the sync ring — W2's 1MB otherwise gates
            # the first main-loop matmul group.
            w2_s = []
            w2_dma_engines = [nc.scalar, nc.scalar, nc.gpsimd, nc.gpsimd]
            for hc in range(HC):
                t = cpool.tile([128, H], F32, name=f"w2s_{hc}")
                w2_dma_engines[hc].dma_start(out=t[:], in_=w2_d[hc * 128 : (hc + 1) * 128, :])
                w2_s.append(t)
            # W3 comes in once as a row and is broadcast on-chip (a
            # partition-stride-0 DMA would re-read the 2KB row 128 times).
            w3row = cpool.tile([1, H], F32, name="w3row")
            nc.gpsimd.dma_start(out=w3row[:], in_=w3_d[:, :].flatten().unsqueeze(0))
            w3b = cpool.tile([128, H], F32, name="w3b")
            nc.gpsimd.partition_broadcast(w3b[:], w3row[:])

            # ---------------- round-copies to the matmul dtype (ACT) ----------
            yT = cpool.tile([DX, B], MMDT, name="yT")
            nc.scalar.copy(out=yT[:], in_=yT_s[:])
            w1y = cpool.tile([DX, H], MMDT, name="w1y")
            nc.scalar.copy(out=w1y[:], in_=w1y_s[:])
            xT = cpool.tile([DX, ROWS], MMDT, name="xT")
            nc.scalar.copy(out=xT[:], in_=xT_s[:])
            w1x = cpool.tile([DX, H], MMDT, name="w1x")
            nc.scalar.copy(out=w1x[:], in_=w1x_s[:])
            w2sb = []
            for hc in range(HC):
                t = cpool.tile([128, H], MMDT, name=f"w2_{hc}")
                nc.scalar.copy(out=t[:], in_=w2_s[hc][:])
                w2sb.append(t)
            if with_b2:
                b2_s = cpool.tile([1, H], F32, name="b2_s")
                nc.sync.dma_start(out=b2_s[:], in_=b2_d[:].unsqueeze(0))
                b2row = cpool.tile([1, H], MMDT, name="b2row")
                nc.scalar.copy(out=b2row[:], in_=b2_s[:])
                ones1 = cpool.tile([1, 128], MMDT, name="ones1")
                nc.vector.memset(ones1[:], 1.0)

            # hxT[hc][h, i] = (x @ W1x).T + b1
            hxT = []
            for hc in range(HC):
                pshx = ppool.tile([128, ROWS], F32, name="pshx", tag="ps")
                nc.tensor.matmul(
                    pshx[:], w1x[:, hc * 128 : (hc + 1) * 128], xT[:], start=True, stop=True
                )
                t = cpool.tile([128, ROWS], F32, name=f"hxT_{hc}")
                nc.vector.tensor_scalar_add(t[:], pshx[:], b1sb[:, hc : hc + 1])
                hxT.append(t)

            # hyT[hc][h, j] = (y @ W1y).T
            hyT = []
            for hc in range(HC):
                pshy = ppool.tile([128, B], F32, name="pshy", tag="ps")
                nc.tensor.matmul(
                    pshy[:], w1y[:, hc * 128 : (hc + 1) * 128], yT[:], start=True, stop=True
                )
                t = cpool.tile([128, B], F32, name=f"hyT_{hc}")
                nc.vector.tensor_copy(out=t[:], in_=pshy[:])
                hyT.append(t)

            # scores accumulated transposed: scoresT[jc][j, i]
            scoresT = [cpool.tile([128, ROWS], F32, name=f"scT_{jc}") for jc in range(HC)]

            # ---------------- main loop over x rows ----------------
            for i in range(ROWS):
                h1T = []
                for hc in range(HC):
                    # ACT: relu(hyT + hx_i). All of stage A lives on ACT so the
                    # DVE has headroom for the stage-C fused reduce — DVE is
                    # otherwise the bottleneck engine (measured 92% busy).
                    t = wpool.tile([128, B], MMDT, name="h1T", tag="h1T", bufs=12)
                    nc.scalar.activation(
                        t[:], hyT[hc][:], Relu, bias=hxT[hc][:, i : i + 1], scale=1.0
                    )
                    h1T.append(t)
                for jc in range(HC):
                    ps2 = ppool.tile([128, B], F32, name="ps2", tag="ps")
                    for hc in range(HC):
                        nc.tensor.matmul(
                            ps2[:],
                            h1T[hc][:, jc * 128 : (jc + 1) * 128],
                            w2sb[hc][:],
                            start=(hc == 0),
                            stop=(hc == HC - 1 and not with_b2),
                        )
                    if with_b2:
                        nc.tensor.matmul(
                            ps2[:], ones1[:], b2row[:], start=False, stop=True
                        )
                    # DVE: scr = relu(ps2) * W3_bcast; scoresT col = sum_k scr
                    scr = wpool.tile([128, B], F32, name="scr", tag="scr", bufs=6)
                    nc.vector.scalar_tensor_tensor(
                        out=scr[:],
                        in0=ps2[:],
                        scalar=0.0,
                        in1=w3b[:],
                        op0=Alu.max,
                        op1=Alu.mult,
                        accum_out=scoresT[jc][:, i : i + 1],
                    )

            # ---------------- store (host un-transposes) ----------------
            for jc in range(HC):
                nc.sync.dma_start(out=out_d[jc, :, :], in_=scoresT[jc][:])

    nc.finalize()  # runs the Bacc pass pipeline (wait splitting etc.)
    return nc


def _get_nc(with_b2: bool, use_f32r: bool = True):
    key = (with_b2, use_f32r)
    if key not in _BUILT:
        _BUILT[key] = _build(with_b2, use_f32r)
    return _BUILT[key]


def _run(inputs: dict, trace: bool = False, use_f32r: bool = True, **spmd_kwargs):
    """Shard, execute on 8 cores, gather. Returns (scores, BassKernelResults)."""
    from concourse.bass_utils import run_bass_kernel_spmd

    x = np.asarray(inputs["x"], dtype=np.float32)
    y = np.asarray(inputs["y"], dtype=np.float32)
    W1 = np.ascontiguousarray(np.asarray(inputs["W1"], dtype=np.float32))
    b1 = np.ascontiguousarray(np.asarray(inputs["b1"], dtype=np.float32))
    W2 = np.ascontiguousarray(np.asarray(inputs["W2"], dtype=np.float32))
    b2 = np.ascontiguousarray(np.asarray(inputs["b2"], dtype=np.float32))
    W3 = np.ascontiguousarray(np.asarray(inputs["W3"], dtype=np.float32))
    b3 = np.asarray(inputs["b3"], dtype=np.float32)

    with_b2 = bool(np.any(b2))
    nc = _get_nc(with_b2, use_f32r)

    yT = np.ascontiguousarray(y.T)
    in_maps = []
    for c in range(N_CORES):
        m = {
            "xT": np.ascontiguousarray(x[c * ROWS : (c + 1) * ROWS].T),
            "yT": yT,
            "W1": W1,
            "b1": b1,
            "W2": W2,
            "W3": W3,
        }
        if with_b2:
            m["b2"] = b2
        in_maps.append(m)

    res = run_bass_kernel_spmd(
        nc, in_maps, core_ids=list(range(N_CORES)), trace=trace, **spmd_kwargs
    )
    # outT[jc, j, i] -> scores_slab[i, jc*128 + j]
    slabs = [
        np.transpose(r["outT"], (2, 0, 1)).reshape(ROWS, B) for r in res.results
    ]
    out = np.concatenate(slabs, axis=0)
    if b3.size and np.any(b3):
        out = out + b3.reshape(-1)[0]
    return np.ascontiguousarray(out.astype(np.float32)), res


def kernel(**inputs) -> np.ndarray:
    out, _ = _run(inputs)
    return out
